# revision 99
# baseline (speedup 1.0000x reference)
"""Trainium2 Bass kernel for multi-head causal self-attention.

Tensor-parallel over 8 NeuronCores: each core owns 2 of the 16 heads.
Per core (SPMD, identical program, different weight shards):
  - QKV projections for its 2 heads (contract over d_model, X^T streamed
    in 512-wide chunks so the fp16 stationary loads hide under matmuls)
  - causal attention (scores kept transposed [k, q], trimmed to the
    causal region; the diagonal blocks accumulate onto a PSUM image of
    the -30000 triangle so no post-exp masking is needed; softmax
    denominator via a ones-column fused into the 129-wide PV matmul and
    folded into the attn transpose as a diag(1/denom) moving operand)
  - output projection partial (its 256 rows of Wo), interleaved into the
    next q-chunk's score loop to keep the PE busy while ACT runs exp
Host: shards weights, pre-transposes X, sums the 8 partials, adds bo.

All matmul operands are fp16 (1 row/cycle at any moving size, half-size
stationary loads); PSUM accumulation stays fp32. Engine budget: PE does
all matmuls; ACT does exp + Q bias; DVE drains K/V/attn and builds the
diag tiles; GpSimd seeds the causal PSUM images and drains half the
output chunks.
"""
import numpy as np
from contextlib import ExitStack

import concourse.bass as bass
import concourse.tile as tile
from concourse import bacc, mybir
from concourse.bass_utils import run_bass_kernel_spmd

# Problem shape (hardcoded per contract)
B, S, D = 2, 2048, 2048
H, DH = 16, 128
N_CORES = 8
HL = H // N_CORES          # heads per core = 2
DHL = HL * DH              # 256
SC = 512                   # s-chunk for projections
NSC = S // SC              # 4 chunks per batch
NKB = S // 128             # 16 key blocks per batch
NQI = S // 512             # 4 q-chunks of 512 per batch
NDC = D // 128             # 16 contraction blocks

F32 = mybir.dt.float32
F16 = mybir.dt.float16
AF = mybir.ActivationFunctionType

_cached_nc = None


def _mm(nc, out, lhsT, rhs, start, stop):
    nc.tensor.matmul(out, lhsT, rhs, start=start, stop=stop)


def build_nc():
    nc = bacc.Bacc("TRN2", target_bir_lowering=False, debug=False, num_devices=N_CORES)

    # chunk-major X^T: [b, s-chunk, partition, d-block, s-in-chunk] so each
    # chunk DMA reads 16KB contiguous per partition
    xt = nc.dram_tensor("xt", [B, NSC, 128, NDC, SC], F16,
                        kind="ExternalInput").ap()
    wq = nc.dram_tensor("wq", [D, DHL], F16, kind="ExternalInput").ap()
    wk = nc.dram_tensor("wk", [D, DHL], F16, kind="ExternalInput").ap()
    wv = nc.dram_tensor("wv", [D, DHL], F16, kind="ExternalInput").ap()
    bqt_d = nc.dram_tensor("bqt", [128, HL], F32, kind="ExternalInput").ap()
    wo = nc.dram_tensor("wo", [HL, 128, D], F16, kind="ExternalInput").ap()
    # [tri(-30000) | zeros]: PSUM seed for causal masking of diag blocks
    trim_d = nc.dram_tensor("trim", [128, 512], F16, kind="ExternalInput").ap()
    vones_d = nc.dram_tensor("vones", [128, 1, 1], F16, kind="ExternalInput").ap()
    ident_d = nc.dram_tensor("ident", [128, 128], F16, kind="ExternalInput").ap()
    out = nc.dram_tensor("out", [B, S, D], F16, kind="ExternalOutput").ap()

    wq_r = wq.rearrange("(n p) d -> p n d", p=128)
    wk_r = wk.rearrange("(n p) d -> p n d", p=128)
    wv_r = wv.rearrange("(n p) d -> p n d", p=128)

    with tile.TileContext(nc) as tc, ExitStack() as ctx:
        pp = ctx.enter_context(tc.tile_pool(name="persist", bufs=1))

        wq_t = pp.tile([128, NDC, DHL], F16)
        wk_t = pp.tile([128, NDC, DHL], F16)
        wv_t = pp.tile([128, NDC, DHL], F16)
        wo_t = pp.tile([128, HL, D], F16)
        bqt = pp.tile([128, HL], F32)
        trim = pp.tile([128, 512], F16)
        ident = pp.tile([128, 128], F16)
        # per-batch Q^T/K^T per head, and [V_h0 | 1 | V_h1] per key block
        qt = pp.tile([128, B, HL, S], F16)
        kt = pp.tile([128, B, HL, S], F16)
        vcat = pp.tile([128, B, NKB, 257], F16)
        gate_t = pp.tile([128, 4], F16)

        nc.sync.dma_start(out=wq_t[:, 0:8], in_=wq_r[:, 0:8])
        nc.scalar.dma_start(out=wq_t[:, 8:16], in_=wq_r[:, 8:16])
        nc.sync.dma_start(out=bqt, in_=bqt_d)
        nc.scalar.dma_start(out=trim, in_=trim_d)
        nc.scalar.dma_start(out=ident, in_=ident_d)
        for b in range(B):
            nc.sync.dma_start(out=vcat[:, b, :, 128:129],
                              in_=vones_d.to_broadcast([128, NKB, 1]))

        xp = ctx.enter_context(tc.tile_pool(name="xtp", bufs=4))
        sm = ctx.enter_context(tc.tile_pool(name="sm", bufs=2))
        xpool = ctx.enter_context(tc.tile_pool(name="expool", bufs=3))
        # single persistent PSUM pool: pq(1) pk(1) sc(2) acc01(1) acc23(1)
        # big[po/psv](2) = 8 banks
        ps = ctx.enter_context(tc.tile_pool(name="ps", bufs=1, space="PSUM"))

        # pending output-projection chunks: (stt_h0, stt_h1, b, qq, dk)
        pending = []

        def emit_outproj(n, floor=0):
            for _ in range(min(n, len(pending) - floor)):
                s0, s1, ob, qq, dk = pending.pop(0)
                po = ps.tile([128, 512], F32, tag="big", bufs=2)
                _mm(nc, po, s0, wo_t[:, 0, dk * 512:(dk + 1) * 512],
                    True, False)
                _mm(nc, po, s1, wo_t[:, 1, dk * 512:(dk + 1) * 512],
                    False, True)
                ot = sm.tile([128, 512], F16, tag="ot", bufs=4)
                # mostly-DVE ot drains: ACT copies interleave with the
                # exp queue, but DVE alone becomes the attention pacer
                if dk == 0:
                    nc.scalar.activation(out=ot, in_=po, func=AF.Copy,
                                         scale=1.0)
                else:
                    nc.vector.tensor_copy(ot, po)
                nc.sync.dma_start(
                    out=out[ob, qq * 128:(qq + 1) * 128,
                            dk * 512:(dk + 1) * 512],
                    in_=ot)

        xtiles = {}

        def issue_xt(b, sc):
            # startup-critical chunks split across BOTH HWDGE rows
            # (sync + scalar) for 2x row bandwidth; batch-0 tail on the
            # sync FIFO, batch-1 on the scalar row (sync carries out
            # writes by then)
            xt_t = xp.tile([128, NDC, SC], F16, tag="xt")
            dma = nc.sync.dma_start if b == 0 else nc.scalar.dma_start
            dma(out=xt_t[:, 0:8], in_=xt[b, sc, :, 0:8])
            dma(out=xt_t[:, 8:16], in_=xt[b, sc, :, 8:16])
            xtiles[(b, sc)] = xt_t

        def emit_qkv_chunk(b, sc):
            xt_t = xtiles.pop((b, sc))
            for h in range(HL):
                psq = ps.tile([128, SC], F32, tag="pq", bufs=1)
                for dc in range(NDC):
                    _mm(nc, psq, wq_t[:, dc, h * 128:(h + 1) * 128],
                        xt_t[:, dc, :], dc == 0, dc == NDC - 1)
                nc.scalar.activation(
                    out=qt[:, b, h, sc * SC:(sc + 1) * SC], in_=psq,
                    func=AF.Identity, bias=bqt[:, h:h + 1], scale=1.0)
                psk = ps.tile([128, SC], F32, tag="pk", bufs=1)
                for dc in range(NDC):
                    _mm(nc, psk, wk_t[:, dc, h * 128:(h + 1) * 128],
                        xt_t[:, dc, :], dc == 0, dc == NDC - 1)
                nc.vector.tensor_copy(
                    kt[:, b, h, sc * SC:(sc + 1) * SC], psk)
            for sb in range(SC // 128):
                kb = sc * (SC // 128) + sb
                psv = ps.tile([128, DHL], F32, tag="big", bufs=2)
                for dc in range(NDC):
                    _mm(nc, psv, xt_t[:, dc, sb * 128:(sb + 1) * 128],
                        wv_t[:, dc, :], dc == 0, dc == NDC - 1)
                nc.vector.tensor_copy(
                    vcat[:, b, kb, 0:128], psv[:, 0:128])
                nc.vector.tensor_copy(
                    vcat[:, b, kb, 129:257], psv[:, 128:256])

        def emit_finalize(b, qi, h, qql, accs, st, a0, dn):
            rc = sm.tile([128, 1], F32, tag="rc")
            nc.vector.reciprocal(rc, accs[qql][:, dn:dn + 1])
            dg = sm.tile([128, 128], F16, tag="dg", bufs=2)
            nc.vector.tensor_scalar_mul(dg, ident, rc)
            an = sm.tile([128, 128], F16, tag="an", bufs=2)
            nc.vector.tensor_copy(an, accs[qql][:, a0:a0 + 128])
            # attn^T with each q column scaled by 1/denom:
            # regular matmul an^T @ diag(rc)
            pst = ps.tile([128, 128], F32, tag="sc", bufs=2)
            _mm(nc, pst, an, dg, True, True)
            stt = sm.tile([128, 128], F16, tag="st", bufs=36)
            nc.vector.tensor_copy(stt, pst)
            st[(h, qql)] = stt
            if h == 1:
                for dk in range(D // 512):
                    pending.append((st[(0, qql)], st[(1, qql)], b,
                                    4 * qi + qql, dk))

        def emit_pv(b, qi, h, pkb, pex, accs, st, vlo, a0, dn):
            for qql in range(4):
                qq = 4 * qi + qql
                if pkb <= qq:
                    # start=True resets the WHOLE bank, so only the
                    # bank's first matmul sets it; the sibling region
                    # accumulates onto the bank-wide zero
                    _mm(nc, accs[qql][:, 0:129],
                        pex[:, qql * 128:(qql + 1) * 128],
                        vcat[:, b, pkb, vlo:vlo + 129],
                        pkb == 0 and qql % 2 == 0, pkb == qq)
            fq = pkb - 4 * qi
            if fq >= 0:
                # this key block closed q-block fq: finalize it now so
                # the softmax/transpose tail overlaps the remaining kbs
                emit_finalize(b, qi, h, fq, accs, st, a0, dn)

        def emit_att_qi(b, qi, pop=True):
            st = {}
            for h in range(HL):
                # vcat col 128 is the ones column: head 0 reads
                # [V_h0 | 1] (denom last), head 1 [1 | V_h1]
                vlo = 0 if h == 0 else 128
                a0 = 0 if h == 0 else 1
                dn = 128 if h == 0 else 0
                accp = [ps.tile([128, 2, 129], F32, tag=f"acc{i}",
                                bufs=1, name=f"acc{i}")
                        for i in range(2)]
                accs = [accp[i // 2][:, i % 2, :] for i in range(4)]
                pend = []
                for kb in range(4 * qi + 4):
                    dq = max(0, (kb - 4 * qi)) * 128
                    diag = kb >= 4 * qi
                    pss = ps.tile([128, 512], F32, tag="sc", bufs=2)
                    ktb = kt[:, b, h, kb * 128:(kb + 1) * 128]
                    if not diag:
                        _mm(nc, pss, ktb,
                            qt[:, b, h, qi * 512:(qi + 1) * 512],
                            True, True)
                    else:
                        # below-diagonal columns first (start=True resets
                        # the bank), then the -30000 triangle seed, then
                        # the diagonal block's scores on top of it
                        if dq + 128 < 512:
                            _mm(nc, pss[:, dq + 128:512], ktb,
                                qt[:, b, h, qi * 512 + dq + 128:
                                   (qi + 1) * 512], True, False)
                        _mm(nc, pss[:, dq:dq + 128], ident,
                            trim[:, 0:128], dq + 128 >= 512, False)
                        _mm(nc, pss[:, dq:dq + 128], ktb,
                            qt[:, b, h, qi * 512 + dq:qi * 512 + dq + 128],
                            False, True)
                    ex = xpool.tile([128, 512], F16, tag="ex", bufs=6)
                    nc.scalar.activation(
                        out=ex[:, dq:512], in_=pss[:, dq:512], func=AF.Exp)
                    # PV at lag 2: the score slot rotation (bufs=2)
                    # already guarantees exp(kb-2) is done before
                    # score(kb) issues, so these PVs never wait on ACT
                    if len(pend) == 2:
                        pkb, pex = pend.pop(0)
                        emit_pv(b, qi, h, pkb, pex, accs, st, vlo, a0, dn)
                    if pop:
                        # reserve 8 chunks of PE filler for the final
                        # attention blocks, which otherwise run dry
                        emit_outproj(1, floor=0 if (b == 1 and qi >= 2)
                                     else 8)
                    pend.append((kb, ex))
                for pkb, pex in pend:
                    emit_pv(b, qi, h, pkb, pex, accs, st, vlo, a0, dn)

        # Schedule: DMA issue decoupled from compute so the sync FIFO
        # carries wq, x00, wk, x01, wv, wo, x02, x03 in need-order; QKV
        # runs one chunk ahead of attention so the next chunk's PSUM
        # drains queue on ACT before attention's exp backlog.
        issue_xt(0, 0)
        nc.sync.dma_start(out=wk_t, in_=wk_r)
        issue_xt(0, 1)
        nc.sync.dma_start(out=wv_t, in_=wv_r)
        nc.scalar.dma_start(out=wo_t[:, 0, :], in_=wo[0])
        nc.scalar.dma_start(out=wo_t[:, 1, :], in_=wo[1])
        emit_qkv_chunk(0, 0)
        issue_xt(0, 2)
        emit_att_qi(0, 0)
        emit_qkv_chunk(0, 1)
        issue_xt(0, 3)
        emit_qkv_chunk(0, 2)
        emit_att_qi(0, 1)
        issue_xt(1, 0)
        emit_qkv_chunk(0, 3)
        emit_att_qi(0, 2)
        issue_xt(1, 1)
        emit_att_qi(0, 3)
        issue_xt(1, 2)
        emit_qkv_chunk(1, 0)
        issue_xt(1, 3)
        emit_qkv_chunk(1, 1)
        emit_att_qi(1, 0)
        emit_qkv_chunk(1, 2)
        emit_att_qi(1, 1)
        emit_qkv_chunk(1, 3)
        emit_att_qi(1, 2)
        emit_att_qi(1, 3)
        emit_outproj(len(pending))

    nc.compile()
    return nc


def _get_nc():
    global _cached_nc
    if _cached_nc is None:
        _cached_nc = build_nc()
    return _cached_nc


def make_in_maps(X, Wq, bq, Wk, bk, Wv, bv, Wo, bo):
    X = np.asarray(X, dtype=np.float32)
    scale = np.float32(1.0 / np.sqrt(DH))
    # [b, sc, p, dc, s']: X[b, sc*512+s', dc*128+p], 16KB contiguous rows
    XT = np.ascontiguousarray(
        np.asarray(X, np.float16).reshape(B, NSC, SC, NDC, 128)
        .transpose(0, 1, 4, 3, 2))
    # col j of diag block allows key row p when j >= p; else -30000 seed
    tri = np.where(np.arange(128)[None, :] >= np.arange(128)[:, None],
                   0.0, -30000.0).astype(np.float16)
    trim = np.zeros((128, 512), np.float16)
    trim[:, 0:128] = tri
    ident = np.eye(128, dtype=np.float16)
    in_maps = []
    for c in range(N_CORES):
        hs = slice(c * DHL, (c + 1) * DHL)
        in_maps.append({
            "xt": XT,
            "wq": np.ascontiguousarray(
                (np.asarray(Wq, np.float32)[:, hs] * scale).astype(np.float16)),
            "wk": np.ascontiguousarray(
                np.asarray(Wk, np.float32)[:, hs].astype(np.float16)),
            "wv": np.ascontiguousarray(
                np.asarray(Wv, np.float32)[:, hs].astype(np.float16)),
            "bqt": np.ascontiguousarray(
                (np.asarray(bq, np.float32)[hs] * scale).reshape(HL, 128).T),
            "wo": np.ascontiguousarray(
                np.asarray(Wo, np.float32)[hs, :].reshape(HL, 128, D)
                .astype(np.float16)),
            "trim": trim,
            "ident": ident,
            "vones": np.ones((128, 1, 1), np.float16),
        })
    return in_maps


def kernel(X, Wq, bq, Wk, bk, Wv, bv, Wo, bo, _trace=False):
    nc = _get_nc()
    in_maps = make_in_maps(X, Wq, bq, Wk, bk, Wv, bv, Wo, bo)
    res = run_bass_kernel_spmd(nc, in_maps, list(range(N_CORES)), trace=_trace)
    acc = res.results[0]["out"].astype(np.float64)
    for c in range(1, N_CORES):
        acc += res.results[c]["out"].astype(np.float64)
    # bv commutes through softmax: sum_k w_k (v_k + bv) = (sum_k w_k v_k) + bv,
    # so the V bias contributes bv @ Wo, folded here with bo.
    acc += np.asarray(bo, np.float64) + (
        np.asarray(bv, np.float64) @ np.asarray(Wo, np.float64))
    out = acc.astype(np.float32)
    if _trace:
        return out, res
    return out
